# revision 54
# baseline (speedup 1.0000x reference)
"""Trainium2 Bass kernel for causal GQA attention (B=1, T=4096, D=2048,
H=16, Hkv=4, Dh=128, RoPE) sharded over 8 NeuronCores.

Sharding: tensor-parallel over heads - each core owns 2 q-heads and the
kv head they share (core c: q-heads {2c, 2c+1}, kv head c//2). Each core
computes its q/k/v projections, RoPE, causal attention and its partial
o_proj contribution y_c = O_c @ Wo_c; the host sums the 8 partials.

v2 on-device dataflow (fp16 operands, f32 PSUM accumulation):
  Phase 1 (projections): per 512-token window, fused q/k/v projection
  into qkvT [d, t] with the two q-heads (resp. k,v) paired into one
  2-bank PSUM tile so a single 1024-wide ACT copy drains both; RoPE
  rotate-half via a PE permutation matmul + DVE mul/add; V transposed
  to natural [t, d] layout on the PE.
  Phase 2 (attention + o_proj): per 512-token q-tile, for each 128-row
  kv tile: S^T for both heads into one 2-bank PSUM tile -> ONE
  1024-wide exp on ACT (fused 1/sqrt(dh) scale, fp16 out) -> causal
  blocks near the diagonal are width-trimmed and masked by a DVE
  multiply with a shared triangular constant -> PV accumulation on the
  PE; the softmax denominator is accumulated on GPSIMD (fp16 adds) and
  reduced across partitions by a single ones-matmul per (q-tile, head)
  instead of one per kv tile. o_proj pieces for the previous q-tile are
  interleaved between attention matmuls so the PE never waits on the
  exp chain.
"""

import sys

sys.path.insert(0, "/opt/trn_rl_repo")

import math
from contextlib import ExitStack

import numpy as np

import concourse.bass as bass
import concourse.tile as tile
from concourse import bacc, mybir
from concourse.bass_utils import run_bass_kernel_spmd
from concourse.masks import make_identity

F16 = mybir.dt.float16
F32 = mybir.dt.float32

B, T, D = 1, 4096, 2048
H, HKV, DH = 16, 4, 128
GROUP = H // HKV
ROPE_BASE = 10000.0
N_CORES = 8
HL = H // N_CORES  # q-heads per core (2)
KC = D // 128      # contraction tiles for projections (16)
NQ = T // 512      # 512-wide q windows (8)
NJ = T // 128      # 128-wide kv tiles (32)
SCALE = 1.0 / math.sqrt(DH)

Exp = mybir.ActivationFunctionType.Exp


def _build(nc):
    xp = nc.dram_tensor("xp", [128, KC, T], F16, kind="ExternalInput").ap()
    wqkv = nc.dram_tensor("wqkv", [128, KC, 4, 128], F16, kind="ExternalInput").ap()
    wo = nc.dram_tensor("wo", [128, HL, D], F16, kind="ExternalInput").ap()
    cos2 = nc.dram_tensor("cos2", [128, T], F16, kind="ExternalInput").ap()
    sinsig = nc.dram_tensor("sinsig", [128, T], F16, kind="ExternalInput").ap()
    perm = nc.dram_tensor("perm", [128, 128], F16, kind="ExternalInput").ap()
    y = nc.dram_tensor("y", [T, D], F16, kind="ExternalOutput").ap()

    with tile.TileContext(nc) as tc, ExitStack() as ctx:
        const = ctx.enter_context(tc.tile_pool(name="const", bufs=1))
        xpool = ctx.enter_context(tc.tile_pool(name="xp", bufs=2))
        # PSUM budget (16 KB/partition): sp 2x4K + ot 1x4K + yp 2x2K
        sp = ctx.enter_context(tc.tile_pool(name="sp", bufs=2, space="PSUM"))
        otp = ctx.enter_context(tc.tile_pool(name="ot", bufs=1, space="PSUM"))
        ypp = ctx.enter_context(tc.tile_pool(name="yp", bufs=2, space="PSUM"))
        ppool = ctx.enter_context(tc.tile_pool(name="pt", bufs=5))
        accp = ctx.enter_context(tc.tile_pool(name="ac", bufs=2))
        otup = ctx.enter_context(tc.tile_pool(name="ou", bufs=2))
        bcp = ctx.enter_context(tc.tile_pool(name="bc", bufs=2))
        swpool = ctx.enter_context(tc.tile_pool(name="sw", bufs=2))
        yrp = ctx.enter_context(tc.tile_pool(name="yr", bufs=2))

        wqkv_sb = const.tile([128, KC, 4, 128], F16, tag="wqkv")
        wo_sb = const.tile([128, HL, D], F16, tag="wo")
        cos_sb = const.tile([128, T], F16, tag="cos")
        sin_sb = const.tile([128, T], F16, tag="sin")
        perm_sb = const.tile([128, 128], F16, tag="perm")
        ident = const.tile([128, 128], F16, tag="ident")
        ones_sb = const.tile([128, 128], F16, tag="ones")
        qkvT = const.tile([128, 4, T], F16, tag="qkvT")  # q0,q1,k,v as [d,t]
        vnat = const.tile([128, NJ, 128], F16, tag="vnat")  # V natural [t, d]
        oT = const.tile([128, HL, T], F16, tag="oT")

        make_identity(nc, ident[:])
        nc.vector.memset(ones_sb[:], 1.0)

        # Warm the PE clock gate and preload the exp activation table
        # while the first input DMAs are in flight.
        tdum = swpool.tile([128, 8], F16, tag="tdum")
        nc.scalar.activation(tdum[:], ones_sb[:, 0:8], Exp)
        for i in range(32):
            wt = ypp.tile([128, 128], F16, tag="yp", name=f"warm{i}")
            nc.tensor.transpose(wt[:], ident[:], ident[:])

        # ---- input DMAs ----
        # window 0's x and the weights are chunked along k and interleaved
        # on the low-latency sync/HWDGE queue so the first projection
        # matmuls can start ~3us in instead of waiting for 4 MB.
        xts = [
            xpool.tile([128, KC, 512], F16, tag="xt", name=f"xt{n}")
            for n in range(NQ)
        ]
        for kc in range(4):
            ks = bass.ts(kc, 4)
            nc.gpsimd.dma_start(xts[0][:, ks, :], xp[:, ks, bass.ts(0, 512)])
            nc.sync.dma_start(wqkv_sb[:, ks], wqkv[:, ks])
        nc.sync.dma_start(cos_sb[:], cos2[:])
        nc.sync.dma_start(sin_sb[:], sinsig[:])
        nc.sync.dma_start(perm_sb[:], perm[:])
        nc.sync.dma_start(wo_sb[:], wo[:])
        for n in range(1, NQ):
            nc.gpsimd.dma_start(xts[n][:], xp[:, :, bass.ts(n, 512)])

        # ---------------- phase 1: projections / RoPE / V transpose -------
        def emit_qpair_rope(n, swq):
            # rotate-half partition swap for the q pair on the PE; its
            # ACT drain + DVE mul/add chain hides under the kv-pair
            # projection matmuls that follow.
            ns = bass.ts(n, 512)
            swq_ps = sp.tile([128, HL, 512], F32, tag="sp", name=f"swqp{n}")
            for h in range(HL):
                nc.tensor.matmul(
                    swq_ps[:, h, :], lhsT=perm_sb[:], rhs=qkvT[:, h, ns],
                    start=True, stop=True,
                )
            nc.scalar.copy(swq[:], swq_ps[:])

        def emit_rope_vtrans(n, swq):
            ns = bass.ts(n, 512)
            # q = q*cos + swap(q)*[-sin; sin] in place on DVE
            for h in range(HL):
                nc.vector.tensor_mul(qkvT[:, h, ns], qkvT[:, h, ns], cos_sb[:, ns])
                nc.vector.tensor_mul(swq[:, h, :], swq[:, h, :], sin_sb[:, ns])
                nc.vector.tensor_add(qkvT[:, h, ns], qkvT[:, h, ns], swq[:, h, :])
            # k swap (yp pool: no pending ACT drain there), then V^T -> V
            # natural [t, d] (PE transpose, fp16 PSUM). Attention on this
            # window's diagonal only reads post-rope k several steps in,
            # so the k chain is off the critical path.
            swk_ps = ypp.tile([128, 512], F32, tag="yp", name=f"swkp{n}")
            nc.tensor.matmul(
                swk_ps[:], lhsT=perm_sb[:], rhs=qkvT[:, 2, ns],
                start=True, stop=True,
            )
            swk = swpool.tile([128, 512], F16, tag="swk", name=f"swk{n}")
            nc.scalar.copy(swk[:], swk_ps[:])
            for jt in range(4 * n, 4 * n + 4):
                vt = ypp.tile([128, 128], F16, tag="yp", name=f"vt{jt}")
                nc.tensor.transpose(vt[:], qkvT[:, 3, bass.ts(jt, 128)], ident[:])
                nc.vector.tensor_copy(vnat[:, jt, :], vt[:])
            nc.vector.tensor_mul(qkvT[:, 2, ns], qkvT[:, 2, ns], cos_sb[:, ns])
            nc.vector.tensor_mul(swk[:], swk[:], sin_sb[:, ns])
            nc.vector.tensor_add(qkvT[:, 2, ns], qkvT[:, 2, ns], swk[:])

        def emit_window(n):
            # fused q/k/v projection; (q0,q1) and (k,v) pairs share a
            # 2-bank PSUM tile so one 1024-wide ACT copy drains both.
            swq = swpool.tile([128, HL, 512], F16, tag="swq", name=f"swq{n}")
            if n == 0:
                # window 0 runs k-outer so matmuls chase the chunked DMAs
                pss = [
                    sp.tile([128, 2, 512], F32, tag="sp", name=f"prj0_{p}")
                    for p in range(2)
                ]
                for k in range(KC):
                    for m in range(4):
                        nc.tensor.matmul(
                            pss[m // 2][:, m % 2, :],
                            lhsT=wqkv_sb[:, k, m, :],
                            rhs=xts[0][:, k, :],
                            start=(k == 0),
                            stop=(k == KC - 1),
                        )
                for pair in range(2):
                    nc.scalar.copy(
                        qkvT[:, 2 * pair:2 * pair + 2, bass.ts(0, 512)], pss[pair][:]
                    )
                emit_qpair_rope(0, swq)
            else:
                for pair in range(2):
                    ps = sp.tile([128, 2, 512], F32, tag="sp", name=f"prj{n}_{pair}")
                    for m in (2 * pair, 2 * pair + 1):
                        for k in range(KC):
                            nc.tensor.matmul(
                                ps[:, m - 2 * pair, :],
                                lhsT=wqkv_sb[:, k, m, :],
                                rhs=xts[n][:, k, :],
                                start=(k == 0),
                                stop=(k == KC - 1),
                            )
                    nc.scalar.copy(
                        qkvT[:, 2 * pair:2 * pair + 2, bass.ts(n, 512)], ps[:]
                    )
                emit_qpair_rope(n, swq)
            emit_rope_vtrans(n, swq)

        # ------------- attention with interleaved projection/o_proj -------
        # Window order: proj(0), rope(0), then per qi: attention(qi) with
        # oproj(qi-1) pieces woven between steps, followed by proj(qi+1) +
        # rope(qi+1) as a dense PE block that covers the normalize drain
        # and lets ACT's exp chain catch up.
        kT = qkvT[:, 2, :]
        yrows = {}

        def emit_oproj_piece(ti, mi, act_copy=None, tail=False):
            if mi == 0:
                yrows[ti] = yrp.tile([128, D], F16, tag="yr", name=f"yr{ti}")
            yp = ypp.tile([128, 512], F32, tag="yp", name=f"yp{ti}_{mi}")
            for h in range(HL):
                nc.tensor.matmul(
                    yp[:],
                    lhsT=oT[:, h, bass.ts(ti, 128)],
                    rhs=wo_sb[:, h, bass.ts(mi, 512)],
                    start=(h == 0),
                    stop=(h == HL - 1),
                )
            if act_copy is None:
                act_copy = (ti % 2 == 0) and mi == 3
            if act_copy:
                nc.scalar.copy(yrows[ti][:, bass.ts(mi, 512)], yp[:])
            else:
                nc.vector.tensor_copy(yrows[ti][:, bass.ts(mi, 512)], yp[:])
            # tail rows stream out in halves on independent HWDGE queues
            # so the final write-back does not serialize behind the last
            # copies on one ring
            if tail and mi == 1:
                nc.sync.dma_start(
                    y[bass.ts(ti, 128), 0:1024], yrows[ti][:, 0:1024]
                )
            if mi == 3:
                if tail:
                    nc.scalar.dma_start(
                        y[bass.ts(ti, 128), 1024:2048], yrows[ti][:, 1024:2048]
                    )
                else:
                    nc.gpsimd.dma_start(y[bass.ts(ti, 128), :], yrows[ti][:])

        emit_window(0)
        for qi in range(NQ):
            njt = 4 * (qi + 1)
            # o_proj pieces for q-window qi-1, interleaved into this
            # window's attention steps; 2 emitted up front so the PE has
            # work while this window's q-RoPE drains on ACT/DVE, 2 held
            # back to cover the denominator drain before lpsum.
            pieces = (
                [(4 * (qi - 1) + tj, mi) for tj in range(4) for mi in range(4)]
                if qi > 0 else []
            )
            n_inline = max(0, len(pieces) - 2)
            pc = 0
            while pc < min(2, n_inline):
                emit_oproj_piece(*pieces[pc])
                pc += 1

            ot = otp.tile([128, HL, 512], F32, tag="ot", name=f"ot{qi}")
            acc = accp.tile([128, HL, 512], F16, tag="ac", name=f"acc{qi}")
            prev = None  # deferred PV: (jt, pt, off, w)
            for jt in range(njt):
                kd = jt - 4 * qi
                off = 128 * max(kd, 0)
                w = 512 - off
                sps = sp.tile([128, HL, 512], F32, tag="sp", name=f"s{qi}_{jt}")
                for h in range(HL):
                    nc.tensor.matmul(
                        sps[:, h, off:512],
                        lhsT=kT[:, bass.ts(jt, 128)],
                        rhs=qkvT[:, h, 512 * qi + off:512 * (qi + 1)],
                        start=True,
                        stop=True,
                    )
                pt = ppool.tile([128, HL, 512], F16, tag="pt", name=f"p{qi}_{jt}")
                nc.scalar.activation(
                    pt[:, :, off:512], sps[:, :, off:512], Exp, scale=SCALE
                )
                if kd >= 0:
                    nc.gpsimd.affine_select(
                        out=pt[:, :, off:512],
                        in_=pt[:, :, off:512],
                        compare_op=mybir.AluOpType.is_ge,
                        fill=0.0,
                        base=0,
                        channel_multiplier=-1,
                        pattern=[[0, HL], [1, w]],
                    )
                if jt == 0:
                    nc.vector.tensor_copy(acc[:], pt[:])
                else:
                    nc.vector.tensor_add(
                        acc[:, :, off:512], acc[:, :, off:512], pt[:, :, off:512]
                    )
                # o_proj piece keeps the PE busy while exp(jt) drains
                while pc < n_inline and pc * njt <= jt * n_inline:
                    emit_oproj_piece(*pieces[pc])
                    pc += 1
                if prev is not None:
                    pjt, ppt, poff, pw = prev
                    for h in range(HL):
                        nc.tensor.matmul(
                            ot[:, h, poff:512],
                            lhsT=vnat[:, pjt, :],
                            rhs=ppt[:, h, poff:512],
                            start=(pjt == 0),
                            stop=False,
                        )
                prev = (jt, pt, off, w)
            pjt, ppt, poff, pw = prev
            for h in range(HL):
                nc.tensor.matmul(
                    ot[:, h, poff:512],
                    lhsT=vnat[:, pjt, :],
                    rhs=ppt[:, h, poff:512],
                    start=(pjt == 0),
                    stop=True,
                )
            # drain the O accumulator unnormalized with one ACT copy so
            # its PSUM banks free ~1us after the last PV, independent of
            # the lpsum/reciprocal chain; the normalize runs off the
            # critical path from SBUF (o_proj only reads oT a window
            # later).
            otu = otup.tile([128, HL, 512], F16, tag="ou", name=f"otu{qi}")
            nc.scalar.copy(otu[:], ot[:])
            while pc < len(pieces):
                emit_oproj_piece(*pieces[pc])
                pc += 1
            # partition-reduce the denominator, then normalize O
            lps = sp.tile([128, HL, 512], F32, tag="sp", name=f"l{qi}")
            for h in range(HL):
                nc.tensor.matmul(
                    lps[:, h, :], lhsT=ones_sb[:], rhs=acc[:, h, :],
                    start=True, stop=True,
                )
            bc = bcp.tile([128, HL, 512], F32, tag="bc", name=f"bc{qi}")
            for h in range(HL):
                nc.vector.reciprocal_approx_fast(bc[:, h, :], lps[:, h, :])
                nc.vector.tensor_mul(
                    oT[:, h, bass.ts(qi, 512)], otu[:, h, :], bc[:, h, :]
                )
            if qi + 1 < NQ:
                emit_window(qi + 1)
        # tail: the last window's o_proj; alternate copy engines so
        # neither ACT nor the backlogged DVE serializes the drain.
        for tj in range(4):
            for mi in range(4):
                emit_oproj_piece(
                    4 * (NQ - 1) + tj, mi, act_copy=(mi % 2 == 0), tail=True
                )


_CACHE = {}


def _get_program():
    if "nc" not in _CACHE:
        nc = bacc.Bacc(
            "TRN2", target_bir_lowering=False, debug=False, num_devices=N_CORES
        )
        _build(nc)
        nc.compile()
        _CACHE["nc"] = nc
    return _CACHE["nc"]


def _rope_tables():
    inv_freq = 1.0 / (ROPE_BASE ** (np.arange(64, dtype=np.float64) / 64))
    ang = np.arange(T, dtype=np.float64)[:, None] * inv_freq[None, :]  # [T, 64]
    cos = np.cos(ang).T  # [64, T]
    sin = np.sin(ang).T
    cos2 = np.concatenate([cos, cos], axis=0).astype(np.float16)
    sinsig = np.concatenate([-sin, sin], axis=0).astype(np.float16)
    return cos2, sinsig


def kernel(x, Wq, Wk, Wv, Wo):
    x = np.asarray(x, dtype=np.float32)
    Wq = np.asarray(Wq, dtype=np.float32)
    Wk = np.asarray(Wk, dtype=np.float32)
    Wv = np.asarray(Wv, dtype=np.float32)
    Wo = np.asarray(Wo, dtype=np.float32)

    # x[t, c] -> xp[p, k, t] = x[t, k*128+p]
    xp = np.ascontiguousarray(
        x.reshape(T, KC, 128).transpose(2, 1, 0)
    ).astype(np.float16)
    cos2, sinsig = _rope_tables()
    d_idx = np.arange(128)
    permm = (d_idx[:, None] == (d_idx[None, :] + 64) % 128).astype(np.float16)

    in_maps = []
    for c in range(N_CORES):
        h0, h1 = 2 * c, 2 * c + 1
        kv = c // 2
        wqkv_c = np.concatenate(
            [
                Wq[:, h0 * DH:(h0 + 1) * DH],
                Wq[:, h1 * DH:(h1 + 1) * DH],
                Wk[:, kv * DH:(kv + 1) * DH],
                Wv[:, kv * DH:(kv + 1) * DH],
            ],
            axis=1,
        )  # [D, 512]
        wqkv_pre = np.ascontiguousarray(
            wqkv_c.reshape(KC, 128, 4, 128).transpose(1, 0, 2, 3)
        ).astype(np.float16)
        wo_pre = np.ascontiguousarray(
            np.stack(
                [Wo[h0 * DH:(h0 + 1) * DH, :], Wo[h1 * DH:(h1 + 1) * DH, :]], axis=0
            ).transpose(1, 0, 2)
        ).astype(np.float16)
        in_maps.append(
            {
                "xp": xp,
                "wqkv": wqkv_pre,
                "wo": wo_pre,
                "cos2": cos2,
                "sinsig": sinsig,
                "perm": permm,
            }
        )

    nc = _get_program()
    res = run_bass_kernel_spmd(nc, in_maps, list(range(N_CORES)))
    out = np.zeros((T, D), dtype=np.float32)
    for c in range(N_CORES):
        out += res.results[c]["y"].astype(np.float32)
    return out.reshape(B, T, D)
